# revision 1
# baseline (speedup 1.0000x reference)
"""Trainium2 Bass kernel for Conv2DCaps with dynamic routing (3 iterations).

Sharding: 8 cores = batch(4) x H-halves(2). Each core gets a 21-row slab of
its batch image (4-row halo overlap) and computes the full routing locally;
the host extracts the exact 15-row output half from each core.

Device layout ("transposed world"): features on partitions, positions on the
free dim. Key tensors per core (grid 21x32 -> P=672 positions):
  u[rc]   [jm=128, (i=8, p=672)]  fp16, SBUF-resident (built once on PE)
  bT      [(rc,i,j)=576 rows -> 5 tiles of [128, 672]] fp32 routing logits
  c       same row layout as bT, fp16 softmax coefficients
All contractions (u-build, iter0-d, sum-over-m, softmax row-sums, expansions)
run on the PE via small/masked matmuls; products run on DVE at fp16 2x mode;
coefficient broadcast (8 rows -> 128 partitions) rides on DMA engines.
"""
import numpy as np

import concourse.bass as bass
import concourse.bacc as bacc
import concourse.mybir as mybir
import concourse.tile as tile
from concourse.bass_utils import run_bass_kernel_spmd

FP32 = mybir.dt.float32
FP16 = mybir.dt.float16
AF = mybir.ActivationFunctionType
OP = mybir.AluOpType

KH = KW = 3
RC = KH * KW                  # 9
B, H, W, CI, NI = 4, 32, 32, 8, 16
CJ, NJ = 8, 16
HJ = WJ = 30
EPS = 1e-7
J_ALL = float(HJ * WJ * CJ)           # 7200
J_ADD = J_ALL - KH * KW * CJ          # 7128
R_NUM = 3

RROWS = 21                    # rows per shard
P = RROWS * W                 # 672 positions
QR, QC = RROWS - 2, 30        # 19 x 30 local outputs
Q = QR * QC                   # 570
PGR, PGC = QR + 4, 34         # padded v grid 23 x 34
NBT = 5                       # bT tiles (576 rows)


class _PhaseStop(Exception):
    pass


def _build_program(phases=99):
    nc = bacc.Bacc("TRN2", target_bir_lowering=False)

    xTn = nc.declare_dram_parameter("xTn", [128, 2 * P], FP16, isOutput=False)
    xT128 = nc.declare_dram_parameter("xT128", [128, P], FP16, isOutput=False)
    wdn = nc.declare_dram_parameter("wdn", [128, 18 * 128], FP16, isOutput=False)
    wd = nc.declare_dram_parameter("wd", [128, RC * 128], FP16, isOutput=False)
    mones = nc.declare_dram_parameter("mones", [128, 8 * 64], FP16, isOutput=False)
    msum = nc.declare_dram_parameter("msum", [128, NBT * 8], FP32, isOutput=False)
    dltr = nc.declare_dram_parameter("dltr", [8, NBT * 128], FP32, isOutput=False)
    ones16 = nc.declare_dram_parameter("ones16", [128, 8], FP32, isOutput=False)
    dltj = nc.declare_dram_parameter("dltj", [8, 128], FP32, isOutput=False)
    emat = nc.declare_dram_parameter("emat", [128, 16 * 128], FP16, isOutput=False)
    vout = nc.declare_dram_parameter("vout", [128, Q], FP32, isOutput=True)

    with tile.TileContext(nc) as tc:
        with (
            tc.tile_pool(name="const", bufs=1) as cpool,
            tc.tile_pool(name="ubig", bufs=1) as upool,
            tc.tile_pool(name="state", bufs=1) as spool,
            tc.tile_pool(name="work", bufs=2) as wpool,
            tc.tile_pool(name="tiny", bufs=3) as tpool,
            tc.tile_pool(name="ex", bufs=1) as epool,
            tc.tile_pool(name="psA", bufs=3, space="PSUM") as ppA,
            tc.tile_pool(name="psB", bufs=1, space="PSUM") as ppB,
        ):
            # ---- load constants / inputs ----
            t_xtn = cpool.tile([128, 2 * P], FP16, tag="xtn")
            t_x128 = cpool.tile([128, P], FP16, tag="x128")
            t_wdn = cpool.tile([128, 18 * 128], FP16, tag="wdn")
            t_wd = cpool.tile([128, RC * 128], FP16, tag="wd")
            t_mo = cpool.tile([128, 8 * 64], FP16, tag="mo")
            t_ms = cpool.tile([128, NBT * 8], FP32, tag="ms")
            t_dr = cpool.tile([8, NBT * 128], FP32, tag="dr")
            t_o16 = cpool.tile([128, 8], FP32, tag="o16")
            t_dj = cpool.tile([8, 128], FP32, tag="dj")
            t_em = cpool.tile([128, 16 * 128], FP16, tag="em")
            t_eps = cpool.tile([128, 1], FP32, tag="eps")
            nc.vector.memset(t_eps[:], EPS)
            for t, src in ((t_xtn, xTn), (t_x128, xT128), (t_wdn, wdn),
                           (t_wd, wd), (t_mo, mones), (t_ms, msum),
                           (t_dr, dltr), (t_o16, ones16), (t_dj, dltj),
                           (t_em, emat)):
                nc.sync.dma_start(t[:], src[:])

            # ---- persistent state ----
            t_u = [upool.tile([128, 8 * P], FP16, tag=f"u{rc}", name=f"u{rc}") for rc in range(RC)]
            t_bt = [spool.tile([128, P], FP32, tag=f"bt{t}", name=f"bt{t}") for t in range(NBT)]
            t_c = [spool.tile([128, P], FP16, tag=f"c{t}", name=f"c{t}") for t in range(NBT)]
            t_vp = [spool.tile([128, PGR, PGC], FP16, tag=f"vp{c}", name=f"vp{c}") for c in range(KW)]
            t_s = spool.tile([128, Q], FP32, tag="s")

            for t in t_bt:
                nc.vector.memset(t[:], 0.0)
            for t in t_vp:
                nc.gpsimd.memset(t[:], 0.0)

            cp_flip = [0]

            def psum_to_sbuf(dst_ap, src_ap):
                # alternate DVE / ACT for PSUM-exit copies
                if cp_flip[0] % 2 == 0:
                    nc.vector.tensor_copy(dst_ap, src_ap)
                else:
                    nc.scalar.activation(dst_ap, src_ap, AF.Copy)
                cp_flip[0] += 1

            # ---- u-build: u[rc][:, i*P:(i+1)*P] = wdn-block.T @ xTn-block ----
            for rc in range(RC):
                for i in range(CI):
                    b = rc * 8 + i
                    rg = 32 * (b % 4)
                    lhsT = t_wdn[rg:rg + 16, 128 * (b // 4):128 * (b // 4) + 128]
                    rhs = t_xtn[rg:rg + 16, P * (i // 4):P * (i // 4) + P]
                    ps = ppA.tile([128, P], FP32, tag="ps")
                    nc.tensor.matmul(ps[:, 0:512], lhsT, rhs[:, 0:512],
                                     start=True, stop=True, tile_position=(rg, 0))
                    nc.tensor.matmul(ps[:, 512:P], lhsT, rhs[:, 512:P],
                                     start=True, stop=True, tile_position=(rg, 0))
                    psum_to_sbuf(t_u[rc][:, i * P:(i + 1) * P], ps[:])

            def s_accum(rc, d_grid_ap):
                """t_s[jm, q] += window of d (d viewed as [128, 21, 32])."""
                r, c = divmod(rc, KW)
                win = d_grid_ap[:, r:r + QR, c:c + QC]
                sview = t_s[:].rearrange("p (a b) -> p a b", b=QC)
                if rc == 0:
                    nc.vector.tensor_copy(sview, win)
                else:
                    nc.vector.tensor_tensor(sview, sview, win, OP.add)

            def squeeze(it):
                """v = squeeze(s); writes vpads (it<2) or vout (it==2)."""
                s2 = wpool.tile([128, Q], FP32, tag="s2")
                nc.vector.tensor_tensor(s2[:], t_s[:], t_s[:], OP.mult)
                sq = ppA.tile([128, P], FP32, tag="ps")
                nc.tensor.matmul(sq[0:8, 0:512], t_o16[:], s2[:, 0:512],
                                 start=True, stop=True)
                nc.tensor.matmul(sq[0:8, 512:Q], t_o16[:], s2[:, 512:Q],
                                 start=True, stop=True)
                sq_ap = sq[0:8, 0:Q]
                t1 = tpool.tile([8, P], FP32, tag="tmp8")
                nc.vector.tensor_scalar_add(t1[:, 0:Q], sq_ap, 1.0)
                sqr = tpool.tile([8, P], FP32, tag="tmp8")
                nc.scalar.activation(sqr[:, 0:Q], sq_ap, AF.Sqrt, bias=t_eps[0:8, :])
                den = tpool.tile([8, P], FP32, tag="tmp8")
                nc.vector.tensor_tensor(den[:, 0:Q], t1[:, 0:Q], sqr[:, 0:Q],
                                        OP.mult)
                rec = tpool.tile([8, P], FP32, tag="tmp8")
                nc.vector.reciprocal(rec[:, 0:Q], den[:, 0:Q])
                f = tpool.tile([8, P], FP32, tag="tmp8")
                nc.vector.tensor_tensor(f[:, 0:Q], sq_ap, rec[:, 0:Q], OP.mult)
                fe = ppA.tile([128, P], FP32, tag="ps")
                nc.tensor.matmul(fe[:, 0:512], t_dj[:], f[:, 0:512],
                                 start=True, stop=True)
                nc.tensor.matmul(fe[:, 512:Q], t_dj[:], f[:, 512:Q],
                                 start=True, stop=True)
                fe_ap = fe[:, 0:Q]
                if it == R_NUM - 1:
                    t_vo = wpool.tile([128, Q], FP32, tag="s2", name="vo2")
                    nc.vector.tensor_tensor(t_vo[:], t_s[:], fe_ap, OP.mult)
                    nc.sync.dma_start(vout[:], t_vo[:])
                else:
                    for c in range(KW):
                        dst = t_vp[c][:, 2:2 + QR, c:c + QC]
                        nc.vector.tensor_tensor(
                            dst,
                            t_s[:].rearrange("p (a b) -> p a b", b=QC),
                            fe_ap.rearrange("p (a b) -> p a b", b=QC),
                            OP.mult)

            def agreement():
                """bT += sum_m u*vp per (rc, i); PE masked-ones matmuls."""
                for rc in range(RC):
                    r, c = divmod(rc, KW)
                    vslice = t_vp[c][:, 2 - r:2 - r + RROWS, 0:W]
                    vb = vslice.unsqueeze(1).broadcast_to([128, 4, RROWS, W])
                    dps = ppA.tile([128, P], FP32, tag="ps")
                    for h in range(2):
                        prod = wpool.tile([128, 4 * P], FP16, tag="big",
                                          name=f"prod{rc}_{h}")
                        useg = t_u[rc][:, h * 4 * P:(h + 1) * 4 * P]
                        nc.vector.tensor_tensor(
                            prod[:].rearrange("p (i a b) -> p i a b", i=4, b=W),
                            useg.rearrange("p (i a b) -> p i a b", i=4, b=W),
                            vb, OP.mult)
                        for ih in range(4):
                            i = h * 4 + ih
                            lhsT = t_mo[:, i * 64:(i + 1) * 64]
                            seg = prod[:, ih * P:(ih + 1) * P]
                            nc.tensor.matmul(dps[0:64, 0:512], lhsT,
                                             seg[:, 0:512],
                                             start=(i == 0), stop=(i == CI - 1))
                            nc.tensor.matmul(dps[0:64, 512:P], lhsT,
                                             seg[:, 512:P],
                                             start=(i == 0), stop=(i == CI - 1))
                    t = rc // 2
                    row = (rc % 2) * 64
                    bslice = t_bt[t][row:row + 64, :]
                    nc.vector.tensor_tensor(bslice, bslice, dps[0:64, :], OP.add)

            def softmax():
                """c = exp(8 bT) * 7200 / (sum + eps + 7128), via PE row-sums."""
                exs = []
                sume = ppB.tile([128, P], FP32, tag="sume")
                for t in range(NBT):
                    ex = epool.tile([128, P], FP32, tag=f"ex{t}", name=f"ex{t}")
                    nc.scalar.activation(ex[:], t_bt[t][:], AF.Exp, scale=8.0)
                    exs.append(ex)
                for t in range(NBT):
                    lhsT = t_ms[:, t * 8:(t + 1) * 8]
                    nc.tensor.matmul(sume[0:8, 0:512], lhsT, exs[t][:, 0:512],
                                     start=(t == 0), stop=(t == NBT - 1))
                    nc.tensor.matmul(sume[0:8, 512:P], lhsT, exs[t][:, 512:P],
                                     start=(t == 0), stop=(t == NBT - 1))
                sumb = tpool.tile([8, P], FP32, tag="tmp8")
                nc.vector.tensor_scalar_add(sumb[:], sume[0:8, :], J_ADD + EPS)
                rec = tpool.tile([8, P], FP32, tag="tmp8")
                nc.vector.reciprocal(rec[:], sumb[:])
                for t in range(NBT):
                    re = ppA.tile([128, P], FP32, tag="ps")
                    lhsT = t_dr[:, t * 128:(t + 1) * 128]
                    nc.tensor.matmul(re[:, 0:512], lhsT, rec[:, 0:512],
                                     start=True, stop=True)
                    nc.tensor.matmul(re[:, 512:P], lhsT, rec[:, 512:P],
                                     start=True, stop=True)
                    nc.vector.tensor_tensor(t_c[t][:], exs[t][:], re[:], OP.mult)

            def d_pass_coeff():
                """s = sum_rc window(sum_i c*u): fp16 products + in-place tree."""
                for rc in range(RC):
                    halves = []
                    for h in range(2):
                        ce = wpool.tile([128, 4 * P], FP16, tag="ce",
                                        name=f"ce{rc}_{h}")
                        for ih in range(4):
                            i = h * 4 + ih
                            g = rc * 64 + i * 8
                            t, row = g // 128, g % 128
                            lhsT = t_em[:, (row // 8) * 128:(row // 8) * 128 + 128]
                            cep = ppA.tile([128, P], FP32, tag="ps",
                                           name=f"cep{rc}_{i}")
                            nc.tensor.matmul(cep[:, 0:512], lhsT,
                                             t_c[t][:, 0:512],
                                             start=True, stop=True)
                            nc.tensor.matmul(cep[:, 512:P], lhsT,
                                             t_c[t][:, 512:P],
                                             start=True, stop=True)
                            nc.scalar.activation(ce[:, ih * P:(ih + 1) * P],
                                                 cep[:], AF.Copy)
                        cu = wpool.tile([128, 4 * P], FP16, tag="big",
                                        name=f"cu{rc}_{h}")
                        useg = t_u[rc][:, h * 4 * P:(h + 1) * 4 * P]
                        nc.vector.tensor_tensor(cu[:], useg, ce[:], OP.mult)
                        halves.append(cu)
                    with nc.allow_low_precision(reason="fp16 routing tree"):
                        cuA, cuB = halves
                        nc.vector.tensor_tensor(cuA[:], cuA[:], cuB[:], OP.add)
                        nc.gpsimd.tensor_tensor(cuA[:, 0:2 * P], cuA[:, 0:2 * P],
                                                cuA[:, 2 * P:4 * P], OP.add)
                        d = wpool.tile([128, P], FP16, tag="d")
                        nc.gpsimd.tensor_tensor(d[:], cuA[:, 0:P],
                                                cuA[:, P:2 * P], OP.add)
                    s_accum(rc, d[:].rearrange("p (a b) -> p a b", b=W))

            if phases < 1:
                raise _PhaseStop
            # ================= iteration 0 =================
            # c == 1 exactly (b=0): d0[rc] = Wd[rc].T @ xT128 on PE (K=128)
            for rc in range(RC):
                ps = ppA.tile([128, P], FP32, tag="ps")
                lhsT = t_wd[:, rc * 128:(rc + 1) * 128]
                nc.tensor.matmul(ps[:, 0:512], lhsT, t_x128[:, 0:512],
                                 start=True, stop=True)
                nc.tensor.matmul(ps[:, 512:P], lhsT, t_x128[:, 512:P],
                                 start=True, stop=True)
                s_accum(rc, ps[:].rearrange("p (a b) -> p a b", b=W))
            squeeze(0)
            if phases < 2:
                raise _PhaseStop
            agreement()
            if phases < 3:
                raise _PhaseStop

            # ================= iterations 1..2 =================
            for it in range(1, R_NUM):
                if phases < 3 + (it - 1) * 4 + 1:
                    raise _PhaseStop
                softmax()
                if phases < 3 + (it - 1) * 4 + 2:
                    raise _PhaseStop
                d_pass_coeff()
                squeeze(it)
                if phases < 3 + (it - 1) * 4 + 3:
                    raise _PhaseStop
                if it < R_NUM - 1:
                    agreement()

    nc.compile()
    return nc


_PROGRAM = None


def _get_program():
    global _PROGRAM
    if _PROGRAM is None:
        _PROGRAM = _build_program()
    return _PROGRAM


def _host_inputs(x, w):
    """Build the 8 per-core input maps."""
    wdn_p = np.zeros((128, 18 * 128), np.float16)
    wd_p = np.zeros((128, RC * 128), np.float16)
    wf = w.reshape(RC, CI, NI, CJ * NJ)              # [rc, i, n, jm]
    for rc in range(RC):
        for i in range(CI):
            b = rc * 8 + i
            wdn_p[32 * (b % 4):32 * (b % 4) + 16,
                  128 * (b // 4):128 * (b // 4) + 128] = wf[rc, i]
            wd_p[i * 16:(i + 1) * 16, rc * 128:(rc + 1) * 128] = wf[rc, i]
    mones = np.zeros((128, 8 * 64), np.float16)
    for i in range(CI):
        for j in range(CJ):
            mones[j * 16:(j + 1) * 16, i * 64 + i * 8 + j] = 1.0
    msum = np.zeros((128, NBT * 8), np.float32)
    dltr = np.zeros((8, NBT * 128), np.float32)
    for g in range(RC * CI * CJ):                    # g = rc*64 + i*8 + j
        i = (g % 64) // 8
        t, r = g // 128, g % 128
        msum[r, t * 8 + i] = 1.0
        dltr[i, t * 128 + r] = J_ALL
    emat = np.zeros((128, 16 * 128), np.float16)
    for e in range(16):
        off = e * 8
        for j in range(CJ):
            for mm in range(NJ):
                emat[off + j, e * 128 + j * 16 + mm] = 1.0
    ones16 = np.zeros((128, 8), np.float32)
    dltj = np.zeros((8, 128), np.float32)
    for j in range(CJ):
        ones16[j * 16:(j + 1) * 16, j] = 1.0
        dltj[j, j * 16:(j + 1) * 16] = 1.0

    shared = dict(wdn=wdn_p, wd=wd_p, mones=mones, msum=msum, dltr=dltr,
                  ones16=ones16, dltj=dltj, emat=emat)

    in_maps = []
    for core in range(8):
        b, half = divmod(core, 2)
        r0 = 0 if half == 0 else H - RROWS
        xs = x[b, r0:r0 + RROWS].astype(np.float16)   # (21, 32, 8, 16)
        xt128 = np.ascontiguousarray(xs.reshape(P, CI * NI).T)
        xtn = np.zeros((128, 2 * P), np.float16)
        for i in range(CI):
            xtn[32 * (i % 4):32 * (i % 4) + 16,
                P * (i // 4):P * (i // 4) + P] = xs[:, :, i, :].reshape(P, NI).T
        m = dict(shared)
        m["xTn"] = xtn
        m["xT128"] = xt128
        in_maps.append(m)
    return in_maps


def _assemble(results):
    out = np.zeros((B, HJ, WJ, CJ, NJ), np.float32)
    for core, res in enumerate(results):
        b, half = divmod(core, 2)
        v = res["vout"].reshape(CJ, NJ, QR, QC).transpose(2, 3, 0, 1)
        if half == 0:
            out[b, 0:15] = v[0:15]
        else:
            out[b, 15:30] = v[4:19]
    return out


def run(x, w, trace=False):
    x = np.asarray(x, np.float32)
    w = np.asarray(w, np.float32)
    nc = _get_program()
    in_maps = _host_inputs(x, w)
    res = run_bass_kernel_spmd(nc, in_maps, core_ids=list(range(8)), trace=trace)
    return _assemble(res.results), res


def kernel(x, w):
    out, _ = run(x, w)
    return out



# revision 8
# speedup vs baseline: 1.1447x; 1.1447x over previous
"""Trainium2 Bass kernel for Conv2DCaps with dynamic routing (3 iterations).

Sharding: 8 cores = batch(4) x H-halves(2). Each core gets a 21-row slab of
its batch image (4-row halo overlap) and computes the full routing locally;
the host extracts the exact 15-row output half from each core.

Device layout ("transposed world"): features on partitions, positions on the
free dim. Key tensors per core (grid 21x32 -> P=672 positions):
  u[rc]   [jm=128, (i=8, p=672)]  fp16, SBUF-resident (built once on PE)
  bT      [(rc,i,j)=576 rows -> 5 tiles of [128, 672]] fp32 routing logits
  c       same row layout as bT, fp16 softmax coefficients
All contractions (u-build, iter0-d, sum-over-m, softmax row-sums, expansions)
run on the PE via small/masked matmuls; products run on DVE at fp16 2x mode;
coefficient broadcast (8 rows -> 128 partitions) rides on DMA engines.
"""
import numpy as np

import concourse.bass as bass
import concourse.bacc as bacc
import concourse.mybir as mybir
import concourse.tile as tile
from concourse.bass_utils import run_bass_kernel_spmd

FP32 = mybir.dt.float32
FP16 = mybir.dt.float16
AF = mybir.ActivationFunctionType
OP = mybir.AluOpType

KH = KW = 3
RC = KH * KW                  # 9
B, H, W, CI, NI = 4, 32, 32, 8, 16
CJ, NJ = 8, 16
HJ = WJ = 30
EPS = 1e-7
J_ALL = float(HJ * WJ * CJ)           # 7200
J_ADD = J_ALL - KH * KW * CJ          # 7128
R_NUM = 3

RROWS = 21                    # rows per shard
P = RROWS * W                 # 672 positions
QR, QC = RROWS - 2, 30        # 19 x 30 local outputs
Q = QR * QC                   # 570
PGR, PGC = QR + 4, 34         # padded v grid 23 x 34
NBT = 5                       # bT tiles (576 rows)


class _PhaseStop(Exception):
    pass


def _build_program(phases=99):
    nc = bacc.Bacc("TRN2", target_bir_lowering=False)

    xTn = nc.declare_dram_parameter("xTn", [128, 2 * P], FP16, isOutput=False)
    xT128 = nc.declare_dram_parameter("xT128", [128, P], FP16, isOutput=False)
    wdn = nc.declare_dram_parameter("wdn", [128, 18 * 128], FP16, isOutput=False)
    wd = nc.declare_dram_parameter("wd", [128, RC * 128], FP16, isOutput=False)
    mones = nc.declare_dram_parameter("mones", [128, 8 * 64], FP16, isOutput=False)
    msum = nc.declare_dram_parameter("msum", [128, NBT * 8], FP32, isOutput=False)
    dltr = nc.declare_dram_parameter("dltr", [8, NBT * 128], FP32, isOutput=False)
    ones16 = nc.declare_dram_parameter("ones16", [128, 8], FP32, isOutput=False)
    dltj = nc.declare_dram_parameter("dltj", [8, 128], FP32, isOutput=False)
    emat = nc.declare_dram_parameter("emat", [128, 16 * 128], FP16, isOutput=False)
    vout = nc.declare_dram_parameter("vout", [128, Q], FP32, isOutput=True)

    with tile.TileContext(nc) as tc:
        with (
            tc.tile_pool(name="const", bufs=1) as cpool,
            tc.tile_pool(name="ubig", bufs=1) as upool,
            tc.tile_pool(name="state", bufs=1) as spool,
            tc.tile_pool(name="work", bufs=2) as wpool,
            tc.tile_pool(name="dtiles", bufs=3) as dpool,
            tc.tile_pool(name="tiny", bufs=3) as tpool,
            tc.tile_pool(name="ex", bufs=1) as epool,
            tc.tile_pool(name="psA", bufs=3, space="PSUM") as ppA,
            tc.tile_pool(name="psB", bufs=1, space="PSUM") as ppB,
        ):
            # ---- load constants / inputs ----
            t_xtn = cpool.tile([128, 2 * P], FP16, tag="xtn")
            t_x128 = cpool.tile([128, P], FP16, tag="x128")
            t_wdn = cpool.tile([128, 18 * 128], FP16, tag="wdn")
            t_wd = cpool.tile([128, RC * 128], FP16, tag="wd")
            t_mo = cpool.tile([128, 8 * 64], FP16, tag="mo")
            t_ms = cpool.tile([128, NBT * 8], FP32, tag="ms")
            t_dr = cpool.tile([8, NBT * 128], FP32, tag="dr")
            t_o16 = cpool.tile([128, 8], FP32, tag="o16")
            t_dj = cpool.tile([8, 128], FP32, tag="dj")
            t_em = cpool.tile([128, 16 * 128], FP16, tag="em")
            t_eps = cpool.tile([128, 1], FP32, tag="eps")
            nc.vector.memset(t_eps[:], EPS)
            for t, src in ((t_xtn, xTn), (t_x128, xT128), (t_wdn, wdn),
                           (t_wd, wd), (t_mo, mones), (t_ms, msum),
                           (t_dr, dltr), (t_o16, ones16), (t_dj, dltj),
                           (t_em, emat)):
                nc.sync.dma_start(t[:], src[:])

            # ---- persistent state ----
            t_u = [upool.tile([128, 8 * P], FP16, tag=f"u{rc}", name=f"u{rc}") for rc in range(RC)]
            t_bt = [spool.tile([128, P], FP32, tag=f"bt{t}", name=f"bt{t}") for t in range(NBT)]
            t_c = [spool.tile([128, P], FP16, tag=f"c{t}", name=f"c{t}") for t in range(NBT)]
            t_vp = [spool.tile([128, PGR, PGC], FP16, tag=f"vp{c}", name=f"vp{c}") for c in range(KW)]
            # three partial window accumulators (shorter serial chains, multi-engine)
            t_sp = [spool.tile([128, Q], FP32, tag=f"s{k}", name=f"s{k}") for k in range(3)]

            for t in t_bt:
                nc.vector.memset(t[:], 0.0)
            for t in t_vp:
                nc.gpsimd.memset(t[:], 0.0)

            def psum_to_sbuf(dst_ap, src_ap):
                nc.any.tensor_copy(dst_ap, src_ap)

            # ---- u-build: u[rc][:, i*P:(i+1)*P] = wdn-block.T @ xTn-block ----
            for rc in range(RC):
                for i in range(CI):
                    b = rc * 8 + i
                    rg = 32 * (b % 4)
                    lhsT = t_wdn[rg:rg + 16, 128 * (b // 4):128 * (b // 4) + 128]
                    rhs = t_xtn[rg:rg + 16, P * (i // 4):P * (i // 4) + P]
                    ps = ppA.tile([128, P], FP32, tag="ps")
                    nc.tensor.matmul(ps[:, 0:512], lhsT, rhs[:, 0:512],
                                     start=True, stop=True, tile_position=(rg, 0))
                    nc.tensor.matmul(ps[:, 512:P], lhsT, rhs[:, 512:P],
                                     start=True, stop=True, tile_position=(rg, 0))
                    psum_to_sbuf(t_u[rc][:, i * P:(i + 1) * P], ps[:])

            def s_accum(rc, d_grid_ap):
                """partial-s[rc%3][jm, q] (+)= window of d (d as [128, 21, 32])."""
                r, c = divmod(rc, KW)
                win = d_grid_ap[:, r:r + QR, c:c + QC]
                sp = t_sp[rc % 3]
                sview = sp[:].rearrange("p (a b) -> p a b", b=QC)
                if rc < 3:
                    nc.any.tensor_copy(sview, win)
                else:
                    nc.any.tensor_tensor(sview, sview, win, OP.add)

            def s_combine():
                """t_sp[0] += t_sp[1] + t_sp[2]; returns the combined tile."""
                nc.any.tensor_tensor(t_sp[1][:], t_sp[1][:], t_sp[2][:], OP.add)
                nc.any.tensor_tensor(t_sp[0][:], t_sp[0][:], t_sp[1][:], OP.add)
                return t_sp[0]

            def squeeze(it):
                """v = squeeze(s); writes vpads (it<2) or vout (it==2)."""
                t_s = s_combine()
                s2 = wpool.tile([128, Q], FP32, tag="s2")
                nc.vector.tensor_tensor(s2[:], t_s[:], t_s[:], OP.mult)
                sq = ppA.tile([128, P], FP32, tag="ps")
                nc.tensor.matmul(sq[0:8, 0:512], t_o16[:], s2[:, 0:512],
                                 start=True, stop=True)
                nc.tensor.matmul(sq[0:8, 512:Q], t_o16[:], s2[:, 512:Q],
                                 start=True, stop=True)
                sq_ap = sq[0:8, 0:Q]
                t1 = tpool.tile([8, P], FP32, tag="tmp8")
                nc.vector.tensor_scalar_add(t1[:, 0:Q], sq_ap, 1.0)
                sqr = tpool.tile([8, P], FP32, tag="tmp8")
                nc.scalar.activation(sqr[:, 0:Q], sq_ap, AF.Sqrt, bias=t_eps[0:8, :])
                den = tpool.tile([8, P], FP32, tag="tmp8")
                nc.vector.tensor_tensor(den[:, 0:Q], t1[:, 0:Q], sqr[:, 0:Q],
                                        OP.mult)
                rec = tpool.tile([8, P], FP32, tag="tmp8")
                nc.vector.reciprocal_approx_fast(rec[:, 0:Q], den[:, 0:Q])
                f = tpool.tile([8, P], FP32, tag="tmp8")
                nc.vector.tensor_tensor(f[:, 0:Q], sq_ap, rec[:, 0:Q], OP.mult)
                fe = ppA.tile([128, P], FP32, tag="ps")
                nc.tensor.matmul(fe[:, 0:512], t_dj[:], f[:, 0:512],
                                 start=True, stop=True)
                nc.tensor.matmul(fe[:, 512:Q], t_dj[:], f[:, 512:Q],
                                 start=True, stop=True)
                fe_ap = fe[:, 0:Q]
                if it == R_NUM - 1:
                    t_vo = wpool.tile([128, Q], FP32, tag="s2", name="vo2")
                    nc.vector.tensor_tensor(t_vo[:], t_s[:], fe_ap, OP.mult)
                    nc.sync.dma_start(vout[:], t_vo[:])
                else:
                    for c in range(KW):
                        dst = t_vp[c][:, 2:2 + QR, c:c + QC]
                        nc.vector.tensor_tensor(
                            dst,
                            t_s[:].rearrange("p (a b) -> p a b", b=QC),
                            fe_ap.rearrange("p (a b) -> p a b", b=QC),
                            OP.mult)

            def agreement():
                """bT += sum_m u*vp per (rc, i); PE masked-ones matmuls."""
                for rc in range(RC):
                    r, c = divmod(rc, KW)
                    vslice = t_vp[c][:, 2 - r:2 - r + RROWS, 0:W]
                    vb = vslice.unsqueeze(1).broadcast_to([128, 4, RROWS, W])
                    dps = ppA.tile([128, P], FP32, tag="ps")
                    for h in range(2):
                        prod = wpool.tile([128, 4 * P], FP16, tag="big",
                                          name=f"prod{rc}_{h}")
                        useg = t_u[rc][:, h * 4 * P:(h + 1) * 4 * P]
                        nc.vector.tensor_tensor(
                            prod[:].rearrange("p (i a b) -> p i a b", i=4, b=W),
                            useg.rearrange("p (i a b) -> p i a b", i=4, b=W),
                            vb, OP.mult)
                        for ih in range(4):
                            i = h * 4 + ih
                            lhsT = t_mo[:, i * 64:(i + 1) * 64]
                            seg = prod[:, ih * P:(ih + 1) * P]
                            nc.tensor.matmul(dps[0:64, 0:512], lhsT,
                                             seg[:, 0:512],
                                             start=(i == 0), stop=(i == CI - 1))
                            nc.tensor.matmul(dps[0:64, 512:P], lhsT,
                                             seg[:, 512:P],
                                             start=(i == 0), stop=(i == CI - 1))
                    t = rc // 2
                    row = (rc % 2) * 64
                    bslice = t_bt[t][row:row + 64, :]
                    nc.any.tensor_tensor(bslice, bslice, dps[0:64, :], OP.add)

            def softmax():
                """c = exp(8 bT) * 7200 / (sum + eps + 7128), via PE row-sums."""
                exs = []
                sume = ppB.tile([128, P], FP32, tag="sume")
                for t in range(NBT):
                    ex = epool.tile([128, P], FP32, tag=f"ex{t}", name=f"ex{t}")
                    nc.scalar.activation(ex[:], t_bt[t][:], AF.Exp, scale=8.0)
                    exs.append(ex)
                for t in range(NBT):
                    lhsT = t_ms[:, t * 8:(t + 1) * 8]
                    nc.tensor.matmul(sume[0:8, 0:512], lhsT, exs[t][:, 0:512],
                                     start=(t == 0), stop=(t == NBT - 1))
                    nc.tensor.matmul(sume[0:8, 512:P], lhsT, exs[t][:, 512:P],
                                     start=(t == 0), stop=(t == NBT - 1))
                sumb = tpool.tile([8, P], FP32, tag="tmp8")
                nc.vector.tensor_scalar_add(sumb[:], sume[0:8, :], J_ADD + EPS)
                rec = tpool.tile([8, P], FP32, tag="tmp8")
                nc.vector.reciprocal_approx_fast(rec[:], sumb[:])
                for t in range(NBT):
                    re = ppA.tile([128, P], FP32, tag="ps")
                    lhsT = t_dr[:, t * 128:(t + 1) * 128]
                    nc.tensor.matmul(re[:, 0:512], lhsT, rec[:, 0:512],
                                     start=True, stop=True)
                    nc.tensor.matmul(re[:, 512:P], lhsT, rec[:, 512:P],
                                     start=True, stop=True)
                    nc.vector.tensor_tensor(t_c[t][:], exs[t][:], re[:], OP.mult)

            def d_pass_coeff():
                """s = sum_rc window(sum_i c*u): fp16 products + in-place tree."""
                for rc in range(RC):
                    halves = []
                    for h in range(2):
                        ce = wpool.tile([128, 4 * P], FP16, tag="ce",
                                        name=f"ce{rc}_{h}")
                        for ih in range(4):
                            i = h * 4 + ih
                            g = rc * 64 + i * 8
                            t, row = g // 128, g % 128
                            lhsT = t_em[:, (row // 8) * 128:(row // 8) * 128 + 128]
                            cep = ppA.tile([128, P], FP32, tag="ps",
                                           name=f"cep{rc}_{i}")
                            nc.tensor.matmul(cep[:, 0:512], lhsT,
                                             t_c[t][:, 0:512],
                                             start=True, stop=True)
                            nc.tensor.matmul(cep[:, 512:P], lhsT,
                                             t_c[t][:, 512:P],
                                             start=True, stop=True)
                            nc.any.tensor_copy(ce[:, ih * P:(ih + 1) * P],
                                               cep[:])
                        cu = wpool.tile([128, 4 * P], FP16, tag="big",
                                        name=f"cu{rc}_{h}")
                        useg = t_u[rc][:, h * 4 * P:(h + 1) * 4 * P]
                        nc.vector.tensor_tensor(cu[:], useg, ce[:], OP.mult)
                        halves.append(cu)
                    with nc.allow_low_precision(reason="fp16 routing tree"):
                        cuA, cuB = halves
                        nc.vector.tensor_tensor(cuA[:], cuA[:], cuB[:], OP.add)
                        nc.any.tensor_tensor(cuA[:, 0:2 * P], cuA[:, 0:2 * P],
                                             cuA[:, 2 * P:4 * P], OP.add)
                        d = dpool.tile([128, P], FP16, tag="d")
                        nc.any.tensor_tensor(d[:], cuA[:, 0:P],
                                             cuA[:, P:2 * P], OP.add)
                    s_accum(rc, d[:].rearrange("p (a b) -> p a b", b=W))

            if phases < 1:
                raise _PhaseStop
            # ================= iteration 0 =================
            # c == 1 exactly (b=0): d0[rc] = Wd[rc].T @ xT128 on PE (K=128)
            for rc in range(RC):
                ps = ppA.tile([128, P], FP32, tag="ps")
                lhsT = t_wd[:, rc * 128:(rc + 1) * 128]
                nc.tensor.matmul(ps[:, 0:512], lhsT, t_x128[:, 0:512],
                                 start=True, stop=True)
                nc.tensor.matmul(ps[:, 512:P], lhsT, t_x128[:, 512:P],
                                 start=True, stop=True)
                s_accum(rc, ps[:].rearrange("p (a b) -> p a b", b=W))
            squeeze(0)
            if phases < 2:
                raise _PhaseStop
            agreement()
            if phases < 3:
                raise _PhaseStop

            # ================= iterations 1..2 =================
            for it in range(1, R_NUM):
                if phases < 3 + (it - 1) * 4 + 1:
                    raise _PhaseStop
                softmax()
                if phases < 3 + (it - 1) * 4 + 2:
                    raise _PhaseStop
                d_pass_coeff()
                squeeze(it)
                if phases < 3 + (it - 1) * 4 + 3:
                    raise _PhaseStop
                if it < R_NUM - 1:
                    agreement()

    nc.compile()
    return nc


_PROGRAM = None


def _get_program():
    global _PROGRAM
    if _PROGRAM is None:
        _PROGRAM = _build_program()
    return _PROGRAM


def _host_inputs(x, w):
    """Build the 8 per-core input maps."""
    wdn_p = np.zeros((128, 18 * 128), np.float16)
    wd_p = np.zeros((128, RC * 128), np.float16)
    wf = w.reshape(RC, CI, NI, CJ * NJ)              # [rc, i, n, jm]
    for rc in range(RC):
        for i in range(CI):
            b = rc * 8 + i
            wdn_p[32 * (b % 4):32 * (b % 4) + 16,
                  128 * (b // 4):128 * (b // 4) + 128] = wf[rc, i]
            wd_p[i * 16:(i + 1) * 16, rc * 128:(rc + 1) * 128] = wf[rc, i]
    mones = np.zeros((128, 8 * 64), np.float16)
    for i in range(CI):
        for j in range(CJ):
            mones[j * 16:(j + 1) * 16, i * 64 + i * 8 + j] = 1.0
    msum = np.zeros((128, NBT * 8), np.float32)
    dltr = np.zeros((8, NBT * 128), np.float32)
    for g in range(RC * CI * CJ):                    # g = rc*64 + i*8 + j
        i = (g % 64) // 8
        t, r = g // 128, g % 128
        msum[r, t * 8 + i] = 1.0
        dltr[i, t * 128 + r] = J_ALL
    emat = np.zeros((128, 16 * 128), np.float16)
    for e in range(16):
        off = e * 8
        for j in range(CJ):
            for mm in range(NJ):
                emat[off + j, e * 128 + j * 16 + mm] = 1.0
    ones16 = np.zeros((128, 8), np.float32)
    dltj = np.zeros((8, 128), np.float32)
    for j in range(CJ):
        ones16[j * 16:(j + 1) * 16, j] = 1.0
        dltj[j, j * 16:(j + 1) * 16] = 1.0

    shared = dict(wdn=wdn_p, wd=wd_p, mones=mones, msum=msum, dltr=dltr,
                  ones16=ones16, dltj=dltj, emat=emat)

    in_maps = []
    for core in range(8):
        b, half = divmod(core, 2)
        r0 = 0 if half == 0 else H - RROWS
        xs = x[b, r0:r0 + RROWS].astype(np.float16)   # (21, 32, 8, 16)
        xt128 = np.ascontiguousarray(xs.reshape(P, CI * NI).T)
        xtn = np.zeros((128, 2 * P), np.float16)
        for i in range(CI):
            xtn[32 * (i % 4):32 * (i % 4) + 16,
                P * (i // 4):P * (i // 4) + P] = xs[:, :, i, :].reshape(P, NI).T
        m = dict(shared)
        m["xTn"] = xtn
        m["xT128"] = xt128
        in_maps.append(m)
    return in_maps


def _assemble(results):
    out = np.zeros((B, HJ, WJ, CJ, NJ), np.float32)
    for core, res in enumerate(results):
        b, half = divmod(core, 2)
        v = res["vout"].reshape(CJ, NJ, QR, QC).transpose(2, 3, 0, 1)
        if half == 0:
            out[b, 0:15] = v[0:15]
        else:
            out[b, 15:30] = v[4:19]
    return out


def run(x, w, trace=False):
    x = np.asarray(x, np.float32)
    w = np.asarray(w, np.float32)
    nc = _get_program()
    in_maps = _host_inputs(x, w)
    res = run_bass_kernel_spmd(nc, in_maps, core_ids=list(range(8)), trace=trace)
    return _assemble(res.results), res


def kernel(x, w):
    out, _ = run(x, w)
    return out

